# revision 5
# baseline (speedup 1.0000x reference)
"""Trainium2 Bass kernel for nn_DynamicWeightProjection (B=4, T=4096, D=4096).

Strategy: data-parallel over the 16384 tokens across 8 NeuronCores (2048 each).
Per core, tokens are processed in groups of 512:
  - X[t, d] (host-cast fp16) is loaded transposed into SBUF via xbar DMA-transpose
    as XT[d, t] tiles.
  - stage 1: H.T = Wcat.T @ X.T on TensorE (fp16 in, fp32 PSUM), Wcat = [dw1 | dd_w]
    pre-scaled by S=16 on host; gelu/tanh on ScalarE descale by 1/S while
    evacuating PSUM -> SBUF.
  - dd rows are PE-transposed back to token-major.
  - stage 2: per 128-token chunk, W[t, (c,i,m)] = H_c.T.T @ qkw_c on TensorE.
  - rmsnorm of w1 (free-axis reduce over m), KW outer products on VectorE with
    the accumulate pass on GpSimd, diagonal += kdd, then DMA out.
All DMAs are HWDGE (sync engine).
"""

import numpy as np

import concourse.bacc as bacc
import concourse.tile as tile
from concourse import mybir
from concourse.bass_utils import run_bass_kernel_spmd
from contextlib import ExitStack

NCORES = 8
B, T, D = 4, 4096, 4096
C, K, M, I = 4, 128, 32, 4
NT = B * T               # 16384 tokens
TC = NT // NCORES        # 2048 tokens per core
TG = 512                 # tokens per group
KD = D // 128            # 32 d-chunks
S = 16.0                 # host weight pre-scale (fp16 subnormal guard)
EPS = 1e-6

F16 = mybir.dt.float16
F32 = mybir.dt.float32
AF = mybir.ActivationFunctionType


def _build(tc_tokens=TC, reps=1):
    nc = bacc.Bacc("TRN2", target_bir_lowering=False, debug=False)
    ngroups = tc_tokens // TG
    jg = TG // 128  # chunks per group

    x_d = nc.dram_tensor("x", [tc_tokens, D], F16, kind="ExternalInput").ap()
    w_d = nc.dram_tensor("wcat", [D, 640], F16, kind="ExternalInput").ap()
    q_d = nc.dram_tensor("qkw", [K, 512], F16, kind="ExternalInput").ap()
    id_d = nc.dram_tensor("ident", [128, 128], F32, kind="ExternalInput").ap()

    w1_d = [nc.dram_tensor(f"w1_{c}", [tc_tokens, 64], F32, kind="ExternalOutput").ap()
            for c in range(4)]
    w2_d = [nc.dram_tensor(f"w2_{c}", [tc_tokens, 64], F32, kind="ExternalOutput").ap()
            for c in range(4)]
    dd_d = [nc.dram_tensor(f"dd_{a}", [tc_tokens, 32], F32, kind="ExternalOutput").ap()
            for a in range(4)]
    kw_d = nc.dram_tensor("kw", [tc_tokens, 2048], F32, kind="ExternalOutput").ap()

    with tile.TileContext(nc) as tc, ExitStack() as ctx:
        cpool = ctx.enter_context(tc.tile_pool(name="const", bufs=1))
        xt_pool = ctx.enter_context(tc.tile_pool(name="xt", bufs=2))
        h_pool = ctx.enter_context(tc.tile_pool(name="h", bufs=2))
        ddt_pool = ctx.enter_context(tc.tile_pool(name="ddt", bufs=2))
        ddg_pool = ctx.enter_context(tc.tile_pool(name="ddg", bufs=2))
        w1g_pool = ctx.enter_context(tc.tile_pool(name="w1g", bufs=2))
        w2g_pool = ctx.enter_context(tc.tile_pool(name="w2g", bufs=2))
        sq_pool = ctx.enter_context(tc.tile_pool(name="sq", bufs=2))
        st_pool = ctx.enter_context(tc.tile_pool(name="st", bufs=2))
        kw_pool = ctx.enter_context(tc.tile_pool(name="kw", bufs=2))
        pp_pool = ctx.enter_context(tc.tile_pool(name="pp", bufs=1))
        ps_pool = ctx.enter_context(tc.tile_pool(name="ps", bufs=2, space="PSUM"))
        wps_pool = ctx.enter_context(tc.tile_pool(name="wps", bufs=2, space="PSUM"))
        tp_pool = ctx.enter_context(tc.tile_pool(name="tp", bufs=2, space="PSUM"))

        wsb = cpool.tile([128, KD, 640], F16)
        nc.sync.dma_start(out=wsb[:], in_=w_d.rearrange("(k p) f -> p k f", p=128))
        qsb = cpool.tile([128, 512], F16)
        nc.sync.dma_start(out=qsb[:], in_=q_d[:])
        idsb = cpool.tile([128, 128], F32)
        nc.sync.dma_start(out=idsb[:], in_=id_d[:])
        epsb = cpool.tile([128, 1], F32)
        nc.vector.memset(epsb[:], EPS * S * S)

        rep_ctx = tc.For_i(0, reps, 1) if reps > 1 else None
        if rep_ctx is not None:
            ctx.enter_context(rep_ctx)
        if True:
            for g in range(ngroups):
                t0 = g * TG
                # --- XT load (xbar transpose) ---
                xt = xt_pool.tile([128, KD, TG], F16)
                for k in range(KD):
                    nc.sync.dma_start(
                        out=xt[:, k, :],
                        in_=x_d[t0:t0 + TG, 128 * k:128 * (k + 1)],
                        transpose=True,
                    )
                # --- stage 1: [f-chunk, t] = Wcat_chunk.T @ XT ---
                hsb = h_pool.tile([128, 4, TG], F16)
                ddt = ddt_pool.tile([128, TG], F32)
                for f in range(5):
                    ps = ps_pool.tile([128, TG], F32)
                    for k in range(KD):
                        nc.tensor.matmul(
                            ps[:],
                            wsb[:, k, 128 * f:128 * (f + 1)],
                            xt[:, k, :],
                            start=(k == 0),
                            stop=(k == KD - 1),
                        )
                    if f < 4:
                        nc.scalar.activation(hsb[:, f, :], ps[:], AF.Gelu, scale=1.0 / S)
                    else:
                        nc.scalar.activation(ddt[:], ps[:], AF.Tanh, scale=1.0 / S)
                # --- dd transpose back to token-major ---
                ddg = ddg_pool.tile([128, jg, 128], F32)
                for j in range(jg):
                    tp = tp_pool.tile([128, 128], F32)
                    nc.tensor.transpose(tp[:], ddt[:, 128 * j:128 * (j + 1)], idsb[:])
                    nc.scalar.copy(ddg[:, j, :], tp[:])
                # --- stage 2 + norm + KW, per 128-token chunk ---
                w1g = w1g_pool.tile([128, jg, 256], F32)
                w2g = w2g_pool.tile([128, jg, 256], F32)
                for j in range(jg):
                    wps = wps_pool.tile([128, 512], F32)
                    for c in range(4):
                        nc.tensor.matmul(
                            wps[:, 128 * c:128 * (c + 1)],
                            hsb[:, c, 128 * j:128 * (j + 1)],
                            qsb[:, 128 * c:128 * (c + 1)],
                            start=True,
                            stop=True,
                        )
                    wv = wps[:].rearrange("p (c i m) -> p c i m", c=4, i=4)
                    w1v = wv[:, :, 0:2, :]   # [128, 4, 2, 32], S-scaled
                    w2v = wv[:, :, 2:4, :]
                    # sum of squares over m per (c, i)
                    wsq = sq_pool.tile([128, 256], F32)
                    nc.scalar.square(
                        wsq[:].rearrange("p (c i m) -> p c i m", c=4, i=2), w1v)
                    ssum = st_pool.tile([128, 8], F32)
                    nc.vector.tensor_reduce(
                        ssum[:],
                        wsq[:].rearrange("p (q m) -> p q m", q=8),
                        axis=mybir.AxisListType.X,
                        op=mybir.AluOpType.add,
                    )
                    # sig = S * sqrt(mean + eps);  rinv = 1 / sig
                    sig = st_pool.tile([128, 8], F32)
                    nc.scalar.activation(sig[:], ssum[:], AF.Sqrt,
                                         scale=1.0 / M, bias=epsb[:])
                    rinv = st_pool.tile([128, 8], F32)
                    nc.vector.reciprocal(rinv[:], sig[:])
                    # w1 normalized (descale folded into rinv)
                    rv = (rinv[:].rearrange("p (c i) -> p c i", c=4)
                          .unsqueeze(3).broadcast_to([128, 4, 2, 32]))
                    w1j = w1g[:, j, :].rearrange("p (c i m) -> p c i m", c=4, i=2)
                    nc.vector.tensor_mul(w1j, w1v, rv)
                    # w2 descaled copy
                    w2j = w2g[:, j, :].rearrange("p (c i m) -> p c i m", c=4, i=2)
                    nc.scalar.mul(w2j, w2v, 1.0 / S)
                    # KW = sum_i kw1n[i,m] * kw2[i,n]  (+ diag kdd)
                    kw = kw_pool.tile([128, 2048], F32)
                    p0 = pp_pool.tile([128, 2048], F32)
                    p1 = pp_pool.tile([128, 2048], F32)
                    w1jj = w1g[:, j, :].rearrange("p (c i m) -> p c i m", c=4, i=2)
                    w2jj = w2g[:, j, :].rearrange("p (c i m) -> p c i m", c=4, i=2)
                    for i in range(2):
                        a = w1jj[:, 1::2, i, :]     # [128, 2, 32] (s, m)
                        b = w2jj[:, 1::2, i, :]     # [128, 2, 32] (s, n)
                        av = a.unsqueeze(3).broadcast_to([128, 2, 32, 32])
                        bv = b.unsqueeze(2).broadcast_to([128, 2, 32, 32])
                        dst = (p0 if i == 0 else p1)
                        nc.vector.tensor_mul(
                            dst[:].rearrange("p (s m n) -> p s m n", s=2, m=32),
                            av, bv)
                    nc.gpsimd.tensor_add(kw[:], p0[:], p1[:])
                    diag = kw[:].rearrange("p (s mn) -> p s mn", s=2)[:, :, ::33]
                    ddj = (ddg[:, j, :].rearrange("p (q h m) -> p q h m", q=2, h=2)
                           [:, :, 1, :])
                    nc.vector.tensor_add(diag, diag, ddj)
                    nc.sync.dma_start(
                        out=kw_d[t0 + 128 * j:t0 + 128 * (j + 1), :], in_=kw[:])
                # --- group outputs ---
                for c in range(4):
                    nc.sync.dma_start(
                        out=w1_d[c][t0:t0 + TG, :].rearrange("(j p) f -> p j f", p=128),
                        in_=w1g[:, :, 64 * c:64 * (c + 1)])
                    nc.sync.dma_start(
                        out=w2_d[c][t0:t0 + TG, :].rearrange("(j p) f -> p j f", p=128),
                        in_=w2g[:, :, 64 * c:64 * (c + 1)])
                for a in range(4):
                    nc.sync.dma_start(
                        out=dd_d[a][t0:t0 + TG, :].rearrange("(j p) f -> p j f", p=128),
                        in_=ddg[:, :, 32 * a:32 * (a + 1)])

    nc.compile()
    return nc


def _prep_inputs(query_vec, dw1, qkw, dd_w):
    X = np.asarray(query_vec, dtype=np.float32).reshape(-1, D)
    Xh = X.astype(np.float16)
    Wcat = np.concatenate(
        [np.asarray(dw1, np.float32).reshape(D, C * K),
         np.asarray(dd_w, np.float32).reshape(D, M * C)], axis=1)
    Wh = (Wcat * S).astype(np.float16)
    Q = np.transpose(np.asarray(qkw, np.float32)[0], (1, 0, 2, 3)).reshape(K, 512)
    Qh = (Q * S).astype(np.float16)
    ident = np.eye(128, dtype=np.float32)
    return Xh, Wh, Qh, ident


_CACHE = {}


def kernel(query_vec, dw1, qkw, dd_w):
    if "nc" not in _CACHE:
        _CACHE["nc"] = _build()
    nc = _CACHE["nc"]

    Xh, Wh, Qh, ident = _prep_inputs(query_vec, dw1, qkw, dd_w)
    in_maps = [
        {"x": Xh[c * TC:(c + 1) * TC], "wcat": Wh, "qkw": Qh, "ident": ident}
        for c in range(NCORES)
    ]
    res = run_bass_kernel_spmd(nc, in_maps, list(range(NCORES)))
    r = res.results

    def cat(name):
        return np.concatenate([r[c][name] for c in range(NCORES)], axis=0)

    w1 = [cat(f"w1_{c}").reshape(B, T, 1, 2, M) for c in range(4)]
    w2 = [cat(f"w2_{c}").reshape(B, T, 1, 2, M) for c in range(4)]
    dd = [cat(f"dd_{a}").reshape(B, T, 1, M) for a in range(4)]
    KW = cat("kw").reshape(B, T, 2, M, M)
    return (w1[0], w2[0], w1[1], w2[1], dd[0], dd[1],
            w1[2], w2[2], w1[3], w2[3], dd[2], dd[3], KW)
